# revision 10
# baseline (speedup 1.0000x reference)
"""GPTQ 4-bit fused dequant + GEMM + bias + residual for Trainium2 (Bass/Tile).

Problem: out[b,s,n] = sum_k x[b,s,k] * W[k,n] + bias[n] + residual[b,s,n]
  where W = (q - z) * s is 4-bit group-quantized (group size 128 along K),
  x: [4, 2048, 4096] f32, packed weight: [512, 4096] int32 (8 nibbles/word).

Sharding: data-parallel over rows (B*S = 8192 -> 1024 rows/core on 8 cores).
Each core reads its x/residual shard plus the (small, packed) full weight,
dequantizes W on-chip, and computes its output shard; no collectives.

The kernel is TensorE-bound (bf16 GEMM floor ~437 us/core), so:
  - 3/4 of K (nibbles j with j%4 != 3) runs as bf16 matmuls;
  - 1/4 of K (j in {3, 7}, i.e. s=3 both u16 halves) runs as fp8e4
    DoubleRow matmuls (2 k-tiles per instruction, ~1.8x bf16 rate).
    Frobenius error of quarter-fp8 is ~1.7e-2 (verified vs the reference,
    gate 2e-2); full fp8 (3.3e-2) would fail.
  - Both parts share one global x64*x64 = 4096 operand scaling (exact in
    bf16: powers of two) so they accumulate into the SAME PSUM bank; the
    epilogue descales by 2^-12 on ScalarE and adds the (bias-folded)
    residual on VectorE.

Host prep does all layout work: x transposed/permuted to [p, t, m] with
k = 1024a + 8p + j for t = 8s + 4h + a (j = s + 4h) in bf16 (*64), plus an
fp8 copy of the s=3 slice in DoubleRow pair layout [p, h, a, m]; packed
weights pre-split into u16 halves, chunk-major; scales broadcast (*64);
bias folded into the residual.

Scheduling: all input loads ride one in-order DMA ring so chunk-0's
weights/scales and the first x piece beat the bulk-x flood (short head);
residual loads go mid-chunk on the scalar ring; s1/s2 dequant sub/mul run
on the otherwise-idle GpSimd engine so the DVE queue (shifts, s0/s3
dequant, next chunk's first group, epilogue adds) never gates TensorE;
j-outer/mt-inner matmul order staggers PSUM bank release at chunk
boundaries, with the per-mt epilogue interleaved into the final DR sweep.
"""

import numpy as np

import concourse.mybir as mybir
import concourse.tile as tile
from concourse import bacc
from concourse.bass_utils import run_bass_kernel_spmd

F32 = mybir.dt.float32
BF16 = mybir.dt.bfloat16
F8 = mybir.dt.float8e4
I32 = mybir.dt.int32
U16 = mybir.dt.uint16

P = 128  # partitions
JT = 8  # nibbles per int32
NIB = 4  # bits per nibble
XS = 64.0  # per-operand scale (total 4096, descaled 2^-12 in epilogue)

# Full problem shape (hardcoded per harness contract)
B, S, K, N = 4, 2048, 4096, 4096
N_CORES = 8
M_FULL = B * S
M_SHARD = M_FULL // N_CORES


def host_prep(input, weight, weight_scales, weight_zeros, bias, residual,
              n=N, nc_chunk=512):
    """Host-side layout transforms (device streams these directly)."""
    import ml_dtypes

    BF = ml_dtypes.bfloat16
    E4 = ml_dtypes.float8_e4m3  # TRN variant: max 240
    A = (K // JT) // P  # 4
    NCH = n // nc_chunk

    # x[m, 1024a + 8p + j] * 64 -> [m, a, p, h, s] (j = s + 4h)
    x64 = np.asarray(input, dtype=np.float32).reshape(M_FULL, K) * np.float32(XS)
    x5 = x64.reshape(M_FULL, A, P, 2, 4)
    # bf16 pieces, tt = 8s + 4h + a (only s = 0..2 is loaded by the kernel)
    xtp = x5.astype(BF).transpose(2, 4, 3, 1, 0)
    xtp = np.ascontiguousarray(xtp.reshape(P, JT * A, M_FULL))
    # fp8 copy of the s=3 slice in DoubleRow pair layout [p, h, a, m]
    xf8 = np.clip(x5[:, :, :, :, 3], -240, 240).astype(E4).transpose(2, 3, 1, 0)
    xf8 = np.ascontiguousarray(xf8)

    # packed words -> u16 halves, chunk-major so each chunk's load is fully
    # contiguous per partition: wsx[ci, p, h, a, nc] = half h of w[128a+p, n]
    w = np.ascontiguousarray(np.asarray(weight, dtype=np.int32))
    wsx = w.view("<u2").reshape(A, P, n, 2).transpose(1, 3, 0, 2)
    wsx = wsx.reshape(P, 2, A, NCH, nc_chunk).transpose(3, 0, 1, 2, 4)
    wsx = np.ascontiguousarray(wsx)

    # scales/zeros broadcast to [ci, p, a, nc]: zb[p, a, n] = z[8a + p//16, n]
    G = weight_scales.shape[0]
    AG = G // JT

    def bcast(t):
        r = t.reshape(AG, JT, n)
        r = np.repeat(r, 16, axis=1)
        r = r.transpose(1, 0, 2)
        r = r.reshape(P, AG, NCH, nc_chunk).transpose(2, 0, 1, 3)
        return np.ascontiguousarray(r.astype(BF))

    zbx = bcast(np.asarray(weight_zeros, dtype=np.float32))
    sbx = bcast(np.asarray(weight_scales, dtype=np.float32) * np.float32(XS))

    # bias folded into residual (exact f32 add)
    res = np.asarray(residual, dtype=np.float32).reshape(M_FULL, n)
    res = res + np.asarray(bias, dtype=np.float32)[None, :]

    return xtp, xf8, wsx, zbx, sbx, np.ascontiguousarray(res)


def build_nc(m_shard=M_SHARD, k=K, n=N, nc_chunk=512):
    """Build the per-core Bass program (SPMD: same program on all cores)."""
    KP = k // JT  # packed rows (512)
    A = KP // P  # 128-row blocks of packed rows (4)
    MT = m_shard // P  # m tiles (8)
    NCH = n // nc_chunk  # n chunks (8)

    nc = bacc.Bacc("TRN2", target_bir_lowering=False)

    xtp = nc.dram_tensor("xtp", [P, JT * A, m_shard], BF16, kind="ExternalInput")
    xf8d = nc.dram_tensor("xf8", [P, 2, A, m_shard], F8, kind="ExternalInput")
    wsx = nc.dram_tensor("wsx", [NCH, P, 2, A, nc_chunk], U16, kind="ExternalInput")
    zbx = nc.dram_tensor("zbx", [NCH, P, A, nc_chunk], BF16, kind="ExternalInput")
    sbx = nc.dram_tensor("sbx", [NCH, P, A, nc_chunk], BF16, kind="ExternalInput")
    res_in = nc.dram_tensor("res", [m_shard, n], F32, kind="ExternalInput")
    out = nc.dram_tensor("out", [m_shard, n], F32, kind="ExternalOutput")

    with tile.TileContext(nc) as tc:
        with (
            tc.tile_pool(name="persist", bufs=1) as persist,
            tc.tile_pool(name="ws", bufs=3) as ws_pool,
            tc.tile_pool(name="qs", bufs=3) as qs_pool,
            tc.tile_pool(name="q", bufs=6) as q_pool,
            tc.tile_pool(name="qd", bufs=2) as qd_pool,
            tc.tile_pool(name="zs", bufs=3) as zs_pool,
            tc.tile_pool(name="res", bufs=10) as res_pool,
            tc.tile_pool(name="osb", bufs=3) as osb_pool,
            tc.tile_pool(name="dsc", bufs=3) as dsc_pool,
            tc.tile_pool(name="psum", bufs=8, space="PSUM") as psum_pool,
        ):
            # ---- all input loads share the sync ring: in-order FIFO gives
            # chunk-0's weights/scales and the first x pieces priority over
            # the bulk-x flood, so the first matmul starts early ----
            def load_chunk(ci):
                ws = ws_pool.tile([P, 2, A, nc_chunk], U16, tag="ws", name=f"ws{ci}")
                nc.sync.dma_start(ws[:], wsx[ci])
                zb = zs_pool.tile([P, A, nc_chunk], BF16, tag="zb", name=f"zb{ci}")
                sb = zs_pool.tile([P, A, nc_chunk], BF16, tag="sb", name=f"sb{ci}")
                nc.sync.dma_start(zb[:], zbx[ci])
                nc.sync.dma_start(sb[:], sbx[ci])
                return ws, zb, sb

            xTs = [
                persist.tile([P, A, m_shard], BF16, tag=f"xT{i}", name=f"xT{i}")
                for i in range(6)  # s = 0..2 only; s = 3 lives in xf8
            ]
            xf8 = persist.tile([P, 2, A, m_shard], F8, tag="xf8", name="xf8")

            chunks = {0: load_chunk(0)}
            nc.sync.dma_start(xTs[0][:], xtp[:, 0:A, :])
            nc.sync.dma_start(xTs[1][:], xtp[:, A : 2 * A, :])
            chunks[1] = load_chunk(1)
            for i in range(2, 6):
                nc.sync.dma_start(xTs[i][:], xtp[:, A * i : A * (i + 1), :])
            nc.sync.dma_start(xf8[:], xf8d[:])

            def deq(ws, zb, sb, s, ci, eng):
                # ((word >> 4s) & 15), both u16 halves at once (the sub
                # below casts u16 -> bf16; bitwise TS ops cannot cast)
                qsb = qs_pool.tile([P, 2, A, nc_chunk], U16, tag="qs",
                                   name=f"qs{ci}_{s}")
                nc.vector.tensor_scalar(
                    out=qsb[:],
                    in0=ws[:],
                    scalar1=NIB * s,
                    scalar2=15,
                    op0=mybir.AluOpType.logical_shift_right,
                    op1=mybir.AluOpType.bitwise_and,
                )
                qjs = []
                for h in range(2):
                    qj = q_pool.tile([P, A, nc_chunk], BF16, tag="q",
                                     name=f"q{ci}_{s}_{h}")
                    eng.tensor_sub(qj[:], qsb[:, h, :, :], zb[:])
                    eng.tensor_mul(qj[:], qj[:], sb[:])
                    qjs.append(qj)
                return qjs

            def deq_f8(ws, zb, sb, ci):
                # s=3 group dequantized into one fp8 tile with the DoubleRow
                # pair (h) on dim1
                qsb = qs_pool.tile([P, 2, A, nc_chunk], U16, tag="qs",
                                   name=f"qs{ci}_3")
                nc.vector.tensor_scalar(
                    out=qsb[:],
                    in0=ws[:],
                    scalar1=NIB * 3,
                    scalar2=15,
                    op0=mybir.AluOpType.logical_shift_right,
                    op1=mybir.AluOpType.bitwise_and,
                )
                qd = qd_pool.tile([P, 2, A, nc_chunk], F8, tag="qd",
                                  name=f"qd{ci}")
                for h in range(2):
                    qj = q_pool.tile([P, A, nc_chunk], BF16, tag="q",
                                     name=f"qt{ci}_{h}")
                    nc.vector.tensor_sub(qj[:], qsb[:, h, :, :], zb[:])
                    nc.vector.tensor_mul(qd[:, h, :, :], qj[:], sb[:])
                return qd

            deq0 = deq(*chunks[0], 0, 0, nc.vector)

            for ci in range(NCH):
                nsl = slice(ci * nc_chunk, (ci + 1) * nc_chunk)
                ws, zb, sb = chunks.pop(ci)
                if ci + 2 < NCH:
                    chunks[ci + 2] = load_chunk(ci + 2)

                ps = [
                    psum_pool.tile([P, nc_chunk], F32, tag="ps", name=f"ps{ci}_{mt}")
                    for mt in range(MT)
                ]
                res_tiles = []

                for s in range(3):
                    qjs = deq0 if s == 0 else deq(ws, zb, sb, s, ci, nc.gpsimd)
                    for h in range(2):
                        for mt in range(MT):
                            for a in range(A):
                                nc.tensor.matmul(
                                    ps[mt][:],
                                    xTs[2 * s + h][:, a, mt * P : (mt + 1) * P],
                                    qjs[h][:, a, :],
                                    start=(s == 0 and h == 0 and a == 0),
                                    stop=False,
                                )
                    if s >= 1:
                        # residual loads mid-chunk (scalar ring): off the
                        # head/boundary critical path, ready for the epilogue
                        for mt in range((s - 1) * MT // 2, s * MT // 2):
                            r = res_pool.tile([P, nc_chunk], F32, tag="res",
                                              name=f"res{ci}_{mt}")
                            nc.scalar.dma_start(
                                r[:], res_in[mt * P : (mt + 1) * P, nsl]
                            )
                            res_tiles.append(r)

                # final k-group (s=3) as fp8 DoubleRow; next chunk's first
                # dequant ahead of the epilogue adds in the DVE queue; per-mt
                # epilogue (ScalarE 2^-12 descale + DVE residual add)
                # interleaved at each mt's stop
                qd = deq_f8(ws, zb, sb, ci)
                if ci + 1 < NCH:
                    deq0 = deq(*chunks[ci + 1], 0, ci + 1, nc.vector)

                for mt in range(MT):
                    for a in range(A):
                        nc.tensor.matmul(
                            ps[mt][:],
                            xf8[:, :, a, mt * P : (mt + 1) * P],
                            qd[:, :, a, :],
                            start=False,
                            stop=(a == A - 1),
                            perf_mode=mybir.MatmulPerfMode.DoubleRow,
                        )
                    t = dsc_pool.tile([P, nc_chunk], F32, tag="dsc")
                    nc.scalar.mul(t[:], ps[mt][:], 2.0 ** -12)
                    osb = osb_pool.tile([P, nc_chunk], F32, tag="osb")
                    nc.vector.tensor_add(osb[:], t[:], res_tiles[mt][:])
                    nc.sync.dma_start(out[mt * P : (mt + 1) * P, nsl], osb[:])

    nc.compile()
    return nc


_NC_CACHE = {}


def _get_nc():
    if "nc" not in _NC_CACHE:
        _NC_CACHE["nc"] = build_nc()
    return _NC_CACHE["nc"]


def kernel(input, weight, weight_scales, weight_zeros, bias, residual, **run_kwargs):
    """Full-input entry point: shards across 8 NeuronCores, returns full output."""
    xtp, xf8, wsx, zbx, sbx, res = host_prep(
        input, weight, weight_scales, weight_zeros, bias, residual
    )
    nc = _get_nc()
    in_maps = []
    for i in range(N_CORES):
        rows = slice(i * M_SHARD, (i + 1) * M_SHARD)
        in_maps.append(
            {
                "xtp": np.ascontiguousarray(xtp[:, :, rows]),
                "xf8": np.ascontiguousarray(xf8[:, :, :, rows]),
                "wsx": wsx,
                "zbx": zbx,
                "sbx": sbx,
                "res": np.ascontiguousarray(res[rows]),
            }
        )
    result = run_bass_kernel_spmd(
        nc, in_maps, core_ids=list(range(N_CORES)), **run_kwargs
    )
    shards = [result.results[i]["out"] for i in range(N_CORES)]
    full = np.concatenate(shards, axis=0).reshape(B, S, N).astype(np.float32)
    if run_kwargs:
        return full, result
    return full


# revision 11
# speedup vs baseline: 1.0127x; 1.0127x over previous
"""GPTQ 4-bit fused dequant + GEMM + bias + residual for Trainium2 (Bass/Tile).

Problem: out[b,s,n] = sum_k x[b,s,k] * W[k,n] + bias[n] + residual[b,s,n]
  where W = (q - z) * s is 4-bit group-quantized (group size 128 along K),
  x: [4, 2048, 4096] f32, packed weight: [512, 4096] int32 (8 nibbles/word).

Sharding: data-parallel over rows (B*S = 8192 -> 1024 rows/core on 8 cores).
Each core reads its x/residual shard plus the (small, packed) full weight,
dequantizes W on-chip, and computes its output shard; no collectives.

The kernel is TensorE-bound (bf16 GEMM floor ~437 us/core), so:
  - 3/4 of K (nibbles j with j%4 != 3) runs as bf16 matmuls;
  - 1/4 of K (j in {3, 7}, i.e. s=3 both u16 halves) runs as fp8e4
    DoubleRow matmuls (2 k-tiles per instruction, ~1.8x bf16 rate).
    Frobenius error of quarter-fp8 is ~1.7e-2 (verified vs the reference,
    gate 2e-2); full fp8 (3.3e-2) would fail.
  - Both parts share one global x64*x64 = 4096 operand scaling (exact in
    bf16: powers of two) so they accumulate into the SAME PSUM bank; the
    epilogue descales by 2^-12 on ScalarE and adds the (bias-folded)
    residual on VectorE.

Host prep does all layout work: x transposed/permuted to [p, t, m] with
k = 1024a + 8p + j for t = 8s + 4h + a (j = s + 4h) in bf16 (*64), plus an
fp8 copy of the s=3 slice in DoubleRow pair layout [p, h, a, m]; packed
weights pre-split into u16 halves, chunk-major; scales broadcast (*64);
bias folded into the residual.

Scheduling: all input loads ride one in-order DMA ring so chunk-0's
weights/scales and the first x piece beat the bulk-x flood (short head);
residual loads go mid-chunk on the scalar ring; s1/s2 dequant sub/mul run
on the otherwise-idle GpSimd engine so the DVE queue (shifts, s0/s3
dequant, next chunk's first group, epilogue adds) never gates TensorE;
j-outer/mt-inner matmul order staggers PSUM bank release at chunk
boundaries, with the per-mt epilogue interleaved into the final DR sweep.
"""

import numpy as np

import concourse.mybir as mybir
import concourse.tile as tile
from concourse import bacc
from concourse.bass_utils import run_bass_kernel_spmd

F32 = mybir.dt.float32
BF16 = mybir.dt.bfloat16
F8 = mybir.dt.float8e4
I32 = mybir.dt.int32
U16 = mybir.dt.uint16

P = 128  # partitions
JT = 8  # nibbles per int32
NIB = 4  # bits per nibble
XS = 64.0  # per-operand scale (total 4096, descaled 2^-12 in epilogue)

# Full problem shape (hardcoded per harness contract)
B, S, K, N = 4, 2048, 4096, 4096
N_CORES = 8
M_FULL = B * S
M_SHARD = M_FULL // N_CORES


def host_prep(input, weight, weight_scales, weight_zeros, bias, residual,
              n=N, nc_chunk=512):
    """Host-side layout transforms (device streams these directly)."""
    import ml_dtypes

    BF = ml_dtypes.bfloat16
    E4 = ml_dtypes.float8_e4m3  # TRN variant: max 240
    A = (K // JT) // P  # 4
    NCH = n // nc_chunk

    # x[m, 1024a + 8p + j] * 64 -> [m, a, p, h, s] (j = s + 4h)
    x64 = np.asarray(input, dtype=np.float32).reshape(M_FULL, K) * np.float32(XS)
    x5 = x64.reshape(M_FULL, A, P, 2, 4)
    # bf16 pieces, tt = 8s + 4h + a (only s = 0..2 is loaded by the kernel)
    xtp = x5.astype(BF).transpose(2, 4, 3, 1, 0)
    xtp = np.ascontiguousarray(xtp.reshape(P, JT * A, M_FULL))
    # fp8 copy of the s=3 slice in DoubleRow pair layout [p, h, a, m]
    xf8 = np.clip(x5[:, :, :, :, 3], -240, 240).astype(E4).transpose(2, 3, 1, 0)
    xf8 = np.ascontiguousarray(xf8)

    # packed words -> u16 halves, chunk-major so each chunk's load is fully
    # contiguous per partition: wsx[ci, p, h, a, nc] = half h of w[128a+p, n]
    w = np.ascontiguousarray(np.asarray(weight, dtype=np.int32))
    wsx = w.view("<u2").reshape(A, P, n, 2).transpose(1, 3, 0, 2)
    wsx = wsx.reshape(P, 2, A, NCH, nc_chunk).transpose(3, 0, 1, 2, 4)
    wsx = np.ascontiguousarray(wsx)

    # scales/zeros broadcast to [ci, p, a, nc]: zb[p, a, n] = z[8a + p//16, n]
    G = weight_scales.shape[0]
    AG = G // JT

    def bcast(t):
        r = t.reshape(AG, JT, n)
        r = np.repeat(r, 16, axis=1)
        r = r.transpose(1, 0, 2)
        r = r.reshape(P, AG, NCH, nc_chunk).transpose(2, 0, 1, 3)
        return np.ascontiguousarray(r.astype(BF))

    zbx = bcast(np.asarray(weight_zeros, dtype=np.float32))
    sbx = bcast(np.asarray(weight_scales, dtype=np.float32) * np.float32(XS))

    # bias folded into residual (exact f32 add)
    res = np.asarray(residual, dtype=np.float32).reshape(M_FULL, n)
    res = res + np.asarray(bias, dtype=np.float32)[None, :]

    return xtp, xf8, wsx, zbx, sbx, np.ascontiguousarray(res)


def build_nc(m_shard=M_SHARD, k=K, n=N, nc_chunk=512):
    """Build the per-core Bass program (SPMD: same program on all cores)."""
    KP = k // JT  # packed rows (512)
    A = KP // P  # 128-row blocks of packed rows (4)
    MT = m_shard // P  # m tiles (8)
    NCH = n // nc_chunk  # n chunks (8)

    nc = bacc.Bacc("TRN2", target_bir_lowering=False)

    xtp = nc.dram_tensor("xtp", [P, JT * A, m_shard], BF16, kind="ExternalInput")
    xf8d = nc.dram_tensor("xf8", [P, 2, A, m_shard], F8, kind="ExternalInput")
    wsx = nc.dram_tensor("wsx", [NCH, P, 2, A, nc_chunk], U16, kind="ExternalInput")
    zbx = nc.dram_tensor("zbx", [NCH, P, A, nc_chunk], BF16, kind="ExternalInput")
    sbx = nc.dram_tensor("sbx", [NCH, P, A, nc_chunk], BF16, kind="ExternalInput")
    res_in = nc.dram_tensor("res", [m_shard, n], F32, kind="ExternalInput")
    out = nc.dram_tensor("out", [m_shard, n], F32, kind="ExternalOutput")

    with tile.TileContext(nc) as tc:
        with (
            tc.tile_pool(name="persist", bufs=1) as persist,
            tc.tile_pool(name="ws", bufs=3) as ws_pool,
            tc.tile_pool(name="qs", bufs=3) as qs_pool,
            tc.tile_pool(name="q", bufs=6) as q_pool,
            tc.tile_pool(name="qd", bufs=2) as qd_pool,
            tc.tile_pool(name="zs", bufs=3) as zs_pool,
            tc.tile_pool(name="res", bufs=10) as res_pool,
            tc.tile_pool(name="osb", bufs=3) as osb_pool,
            tc.tile_pool(name="dsc", bufs=3) as dsc_pool,
            tc.tile_pool(name="psum", bufs=8, space="PSUM") as psum_pool,
        ):
            # ---- all input loads share the sync ring: in-order FIFO gives
            # chunk-0's weights/scales and the first x pieces priority over
            # the bulk-x flood, so the first matmul starts early ----
            def load_chunk(ci):
                ws = ws_pool.tile([P, 2, A, nc_chunk], U16, tag="ws", name=f"ws{ci}")
                nc.sync.dma_start(ws[:], wsx[ci])
                zb = zs_pool.tile([P, A, nc_chunk], BF16, tag="zb", name=f"zb{ci}")
                sb = zs_pool.tile([P, A, nc_chunk], BF16, tag="sb", name=f"sb{ci}")
                nc.sync.dma_start(zb[:], zbx[ci])
                nc.sync.dma_start(sb[:], sbx[ci])
                return ws, zb, sb

            xTs = [
                persist.tile([P, A, m_shard], BF16, tag=f"xT{i}", name=f"xT{i}")
                for i in range(6)  # s = 0..2 only; s = 3 lives in xf8
            ]
            xf8 = persist.tile([P, 2, A, m_shard], F8, tag="xf8", name="xf8")

            chunks = {0: load_chunk(0)}
            nc.sync.dma_start(xTs[0][:], xtp[:, 0:A, :])
            nc.sync.dma_start(xTs[1][:], xtp[:, A : 2 * A, :])
            chunks[1] = load_chunk(1)
            for i in range(2, 6):
                nc.sync.dma_start(xTs[i][:], xtp[:, A * i : A * (i + 1), :])
            nc.sync.dma_start(xf8[:], xf8d[:])

            def deq(ws, zb, sb, s, ci, eng):
                # ((word >> 4s) & 15), both u16 halves at once (the sub
                # below casts u16 -> bf16; bitwise TS ops cannot cast)
                qsb = qs_pool.tile([P, 2, A, nc_chunk], U16, tag="qs",
                                   name=f"qs{ci}_{s}")
                nc.vector.tensor_scalar(
                    out=qsb[:],
                    in0=ws[:],
                    scalar1=NIB * s,
                    scalar2=15,
                    op0=mybir.AluOpType.logical_shift_right,
                    op1=mybir.AluOpType.bitwise_and,
                )
                qjs = []
                for h in range(2):
                    qj = q_pool.tile([P, A, nc_chunk], BF16, tag="q",
                                     name=f"q{ci}_{s}_{h}")
                    eng.tensor_sub(qj[:], qsb[:, h, :, :], zb[:])
                    eng.tensor_mul(qj[:], qj[:], sb[:])
                    qjs.append(qj)
                return qjs

            def deq_f8(ws, zb, sb, ci):
                # s=3 group dequantized into one fp8 tile with the DoubleRow
                # pair (h) on dim1
                qsb = qs_pool.tile([P, 2, A, nc_chunk], U16, tag="qs",
                                   name=f"qs{ci}_3")
                nc.vector.tensor_scalar(
                    out=qsb[:],
                    in0=ws[:],
                    scalar1=NIB * 3,
                    scalar2=15,
                    op0=mybir.AluOpType.logical_shift_right,
                    op1=mybir.AluOpType.bitwise_and,
                )
                qd = qd_pool.tile([P, 2, A, nc_chunk], F8, tag="qd",
                                  name=f"qd{ci}")
                for h in range(2):
                    qj = q_pool.tile([P, A, nc_chunk], BF16, tag="q",
                                     name=f"qt{ci}_{h}")
                    nc.vector.tensor_sub(qj[:], qsb[:, h, :, :], zb[:])
                    nc.vector.tensor_mul(qd[:, h, :, :], qj[:], sb[:])
                return qd

            deq0 = deq(*chunks[0], 0, 0, nc.vector)

            for ci in range(NCH):
                nsl = slice(ci * nc_chunk, (ci + 1) * nc_chunk)
                ws, zb, sb = chunks.pop(ci)
                if ci + 2 < NCH:
                    chunks[ci + 2] = load_chunk(ci + 2)

                ps = [
                    psum_pool.tile([P, nc_chunk], F32, tag="ps", name=f"ps{ci}_{mt}")
                    for mt in range(MT)
                ]
                res_tiles = []

                for s in range(3):
                    qjs = deq0 if s == 0 else deq(ws, zb, sb, s, ci, nc.vector)
                    for h in range(2):
                        for mt in range(MT):
                            for a in range(A):
                                nc.tensor.matmul(
                                    ps[mt][:],
                                    xTs[2 * s + h][:, a, mt * P : (mt + 1) * P],
                                    qjs[h][:, a, :],
                                    start=(s == 0 and h == 0 and a == 0),
                                    stop=False,
                                )
                    if s >= 1:
                        # residual loads mid-chunk (scalar ring): off the
                        # head/boundary critical path, ready for the epilogue
                        for mt in range((s - 1) * MT // 2, s * MT // 2):
                            r = res_pool.tile([P, nc_chunk], F32, tag="res",
                                              name=f"res{ci}_{mt}")
                            nc.scalar.dma_start(
                                r[:], res_in[mt * P : (mt + 1) * P, nsl]
                            )
                            res_tiles.append(r)

                # final k-group (s=3) as fp8 DoubleRow; next chunk's first
                # dequant ahead of the epilogue adds in the DVE queue; per-mt
                # epilogue (ScalarE 2^-12 descale + DVE residual add)
                # interleaved at each mt's stop
                qd = deq_f8(ws, zb, sb, ci)
                if ci + 1 < NCH:
                    deq0 = deq(*chunks[ci + 1], 0, ci + 1, nc.vector)

                for mt in range(MT):
                    for a in range(A):
                        nc.tensor.matmul(
                            ps[mt][:],
                            xf8[:, :, a, mt * P : (mt + 1) * P],
                            qd[:, :, a, :],
                            start=False,
                            stop=(a == A - 1),
                            perf_mode=mybir.MatmulPerfMode.DoubleRow,
                        )
                    t = dsc_pool.tile([P, nc_chunk], F32, tag="dsc")
                    nc.scalar.mul(t[:], ps[mt][:], 2.0 ** -12)
                    osb = osb_pool.tile([P, nc_chunk], F32, tag="osb")
                    nc.vector.tensor_add(osb[:], t[:], res_tiles[mt][:])
                    nc.sync.dma_start(out[mt * P : (mt + 1) * P, nsl], osb[:])

    nc.compile()
    return nc


_NC_CACHE = {}


def _get_nc():
    if "nc" not in _NC_CACHE:
        _NC_CACHE["nc"] = build_nc()
    return _NC_CACHE["nc"]


def kernel(input, weight, weight_scales, weight_zeros, bias, residual, **run_kwargs):
    """Full-input entry point: shards across 8 NeuronCores, returns full output."""
    xtp, xf8, wsx, zbx, sbx, res = host_prep(
        input, weight, weight_scales, weight_zeros, bias, residual
    )
    nc = _get_nc()
    in_maps = []
    for i in range(N_CORES):
        rows = slice(i * M_SHARD, (i + 1) * M_SHARD)
        in_maps.append(
            {
                "xtp": np.ascontiguousarray(xtp[:, :, rows]),
                "xf8": np.ascontiguousarray(xf8[:, :, :, rows]),
                "wsx": wsx,
                "zbx": zbx,
                "sbx": sbx,
                "res": np.ascontiguousarray(res[rows]),
            }
        )
    result = run_bass_kernel_spmd(
        nc, in_maps, core_ids=list(range(N_CORES)), **run_kwargs
    )
    shards = [result.results[i]["out"] for i in range(N_CORES)]
    full = np.concatenate(shards, axis=0).reshape(B, S, N).astype(np.float32)
    if run_kwargs:
        return full, result
    return full


# revision 13
# speedup vs baseline: 1.0222x; 1.0094x over previous
"""GPTQ 4-bit fused dequant + GEMM + bias + residual for Trainium2 (Bass/Tile).

Problem: out[b,s,n] = sum_k x[b,s,k] * W[k,n] + bias[n] + residual[b,s,n]
  where W = (q - z) * s is 4-bit group-quantized (group size 128 along K),
  x: [4, 2048, 4096] f32, packed weight: [512, 4096] int32 (8 nibbles/word).

Sharding: data-parallel over rows (B*S = 8192 -> 1024 rows/core on 8 cores).
Each core reads its x/residual shard plus the (small, packed) full weight,
dequantizes W on-chip, and computes its output shard; no collectives.

The kernel is TensorE-bound: the bf16 GEMM floor is ~437 us/core and the
schedule keeps the PE array back-to-back (measured 216 ns per 128x128x512
matmul = the N/2.4GHz streaming floor). fp8 DoubleRow was tried and rejected:
quarter-K fp8 passes accuracy (1.67e-2 < 2e-2; more fp8 fails) but its power
draw clock-throttles the whole chip by 1.2x, a net loss at any allowed mix.

Host prep does all layout work so the device only streams:
  - x transposed/permuted/bf16-cast to [p, t, m] with k = 1024a + 8p + j for
    t = 8s + 4h + a (j = s + 4h), making the packed-word unpacking full-width
    with both matmul operands on the same k ordering; no on-chip transpose.
  - packed weights pre-split into u16 halves, chunk-major (fully contiguous
    per-chunk loads); scales/zeros broadcast to the partition layout in bf16.
  - bias folded into the residual (exact f32 add).

Scheduling: all input loads ride one in-order DMA ring so chunk-0's
weights/scales and the leading x pieces beat the bulk-x flood (short head);
residual loads go mid-chunk on the scalar ring; the DVE queue runs shifts +
sub/mul dequant with the next chunk's first group issued ahead of the
epilogue adds; j-outer/mt-inner matmul order staggers PSUM bank release at
chunk boundaries, with the per-mt epilogue interleaved into the final sweep.
"""

import numpy as np

import concourse.mybir as mybir
import concourse.tile as tile
from concourse import bacc
from concourse.bass_utils import run_bass_kernel_spmd

F32 = mybir.dt.float32
BF16 = mybir.dt.bfloat16
I32 = mybir.dt.int32
U16 = mybir.dt.uint16

P = 128  # partitions
JT = 8  # nibbles per int32
NIB = 4  # bits per nibble

# Full problem shape (hardcoded per harness contract)
B, S, K, N = 4, 2048, 4096, 4096
N_CORES = 8
M_FULL = B * S
M_SHARD = M_FULL // N_CORES


def host_prep(input, weight, weight_scales, weight_zeros, bias, residual,
              n=N, nc_chunk=512):
    """Host-side layout transforms (device streams these directly)."""
    import ml_dtypes

    BF = ml_dtypes.bfloat16
    A = (K // JT) // P  # 4
    NCH = n // nc_chunk

    # x[m, 1024a + 8p + j] -> xtp[p, 8s + 4h + a, m], j = s + 4h, bf16
    xf = np.asarray(input, dtype=np.float32).reshape(M_FULL, K)
    x5 = xf.reshape(M_FULL, A, P, 2, 4)
    xtp = x5.astype(BF).transpose(2, 4, 3, 1, 0)
    xtp = np.ascontiguousarray(xtp.reshape(P, JT * A, M_FULL))

    # packed words -> u16 halves, chunk-major so each chunk's load is fully
    # contiguous per partition: wsx[ci, p, h, a, nc] = half h of w[128a+p, n]
    w = np.ascontiguousarray(np.asarray(weight, dtype=np.int32))
    wsx = w.view("<u2").reshape(A, P, n, 2).transpose(1, 3, 0, 2)
    wsx = wsx.reshape(P, 2, A, NCH, nc_chunk).transpose(3, 0, 1, 2, 4)
    wsx = np.ascontiguousarray(wsx)

    # scales/zeros broadcast to [ci, p, a, nc]: zb[p, a, n] = z[8a + p//16, n]
    G = weight_scales.shape[0]
    AG = G // JT

    def bcast(t):
        r = t.reshape(AG, JT, n)
        r = np.repeat(r, 16, axis=1)
        r = r.transpose(1, 0, 2)
        r = r.reshape(P, AG, NCH, nc_chunk).transpose(2, 0, 1, 3)
        return np.ascontiguousarray(r.astype(BF))

    zbx = bcast(np.asarray(weight_zeros, dtype=np.float32))
    sbx = bcast(np.asarray(weight_scales, dtype=np.float32))

    # bias folded into residual (exact f32 add)
    res = np.asarray(residual, dtype=np.float32).reshape(M_FULL, n)
    res = res + np.asarray(bias, dtype=np.float32)[None, :]

    return xtp, wsx, zbx, sbx, np.ascontiguousarray(res)


def build_nc(m_shard=M_SHARD, k=K, n=N, nc_chunk=512):
    """Build the per-core Bass program (SPMD: same program on all cores)."""
    KP = k // JT  # packed rows (512)
    A = KP // P  # 128-row blocks of packed rows (4)
    MT = m_shard // P  # m tiles (8)
    NCH = n // nc_chunk  # n chunks (8)

    nc = bacc.Bacc("TRN2", target_bir_lowering=False)

    xtp = nc.dram_tensor("xtp", [P, JT * A, m_shard], BF16, kind="ExternalInput")
    wsx = nc.dram_tensor("wsx", [NCH, P, 2, A, nc_chunk], U16, kind="ExternalInput")
    zbx = nc.dram_tensor("zbx", [NCH, P, A, nc_chunk], BF16, kind="ExternalInput")
    sbx = nc.dram_tensor("sbx", [NCH, P, A, nc_chunk], BF16, kind="ExternalInput")
    res_in = nc.dram_tensor("res", [m_shard, n], F32, kind="ExternalInput")
    out = nc.dram_tensor("out", [m_shard, n], F32, kind="ExternalOutput")

    with tile.TileContext(nc) as tc:
        with (
            tc.tile_pool(name="persist", bufs=1) as persist,
            tc.tile_pool(name="ws", bufs=3) as ws_pool,
            tc.tile_pool(name="qs", bufs=3) as qs_pool,
            tc.tile_pool(name="q", bufs=6) as q_pool,
            tc.tile_pool(name="zs", bufs=3) as zs_pool,
            tc.tile_pool(name="res", bufs=12) as res_pool,
            tc.tile_pool(name="osb", bufs=3) as osb_pool,
            tc.tile_pool(name="psum", bufs=8, space="PSUM") as psum_pool,
        ):
            # ---- all input loads share the sync ring: in-order FIFO gives
            # chunk-0's weights/scales and the leading x pieces priority over
            # the bulk-x flood, so the first matmul starts early ----
            def load_chunk(ci):
                ws = ws_pool.tile([P, 2, A, nc_chunk], U16, tag="ws", name=f"ws{ci}")
                nc.sync.dma_start(ws[:], wsx[ci])
                zb = zs_pool.tile([P, A, nc_chunk], BF16, tag="zb", name=f"zb{ci}")
                sb = zs_pool.tile([P, A, nc_chunk], BF16, tag="sb", name=f"sb{ci}")
                nc.sync.dma_start(zb[:], zbx[ci])
                nc.sync.dma_start(sb[:], sbx[ci])
                return ws, zb, sb

            xTs = [
                persist.tile([P, A, m_shard], BF16, tag=f"xT{i}", name=f"xT{i}")
                for i in range(JT)
            ]

            chunks = {0: load_chunk(0)}
            nc.sync.dma_start(xTs[0][:], xtp[:, 0:A, :])
            nc.sync.dma_start(xTs[1][:], xtp[:, A : 2 * A, :])
            chunks[1] = load_chunk(1)
            for i in range(2, JT):
                nc.sync.dma_start(xTs[i][:], xtp[:, A * i : A * (i + 1), :])

            def deq(ws, zb, sb, s, ci):
                # ((word >> 4s) & 15), both u16 halves at once (the sub
                # below casts u16 -> bf16; bitwise TS ops cannot cast)
                qsb = qs_pool.tile([P, 2, A, nc_chunk], U16, tag="qs",
                                   name=f"qs{ci}_{s}")
                nc.vector.tensor_scalar(
                    out=qsb[:],
                    in0=ws[:],
                    scalar1=NIB * s,
                    scalar2=15,
                    op0=mybir.AluOpType.logical_shift_right,
                    op1=mybir.AluOpType.bitwise_and,
                )
                qjs = []
                for h in range(2):
                    qj = q_pool.tile([P, A, nc_chunk], BF16, tag="q",
                                     name=f"q{ci}_{s}_{h}")
                    nc.vector.tensor_sub(qj[:], qsb[:, h, :, :], zb[:])
                    nc.vector.tensor_mul(qj[:], qj[:], sb[:])
                    qjs.append(qj)
                return qjs

            deq0 = deq(*chunks[0], 0, 0)

            for ci in range(NCH):
                nsl = slice(ci * nc_chunk, (ci + 1) * nc_chunk)
                ws, zb, sb = chunks.pop(ci)
                if ci + 2 < NCH:
                    chunks[ci + 2] = load_chunk(ci + 2)

                ps = [
                    psum_pool.tile([P, nc_chunk], F32, tag="ps", name=f"ps{ci}_{mt}")
                    for mt in range(MT)
                ]
                res_tiles = []

                for s in range(3):
                    qjs = deq0 if s == 0 else deq(ws, zb, sb, s, ci)
                    for h in range(2):
                        for mt in range(MT):
                            for a in range(A):
                                nc.tensor.matmul(
                                    ps[mt][:],
                                    xTs[2 * s + h][:, a, mt * P : (mt + 1) * P],
                                    qjs[h][:, a, :],
                                    start=(s == 0 and h == 0 and a == 0),
                                    stop=False,
                                )
                    if s >= 1:
                        # residual loads mid-chunk (scalar ring): off the
                        # head/boundary critical path, ready for the epilogue
                        for mt in range((s - 1) * MT // 2, s * MT // 2):
                            r = res_pool.tile([P, nc_chunk], F32, tag="res",
                                              name=f"res{ci}_{mt}")
                            nc.scalar.dma_start(
                                r[:], res_in[mt * P : (mt + 1) * P, nsl]
                            )
                            res_tiles.append(r)

                # last k-group: dequant, then next chunk's first dequant
                # (ahead of the epilogue adds in the DVE queue), then matmuls
                # with the per-mt epilogue interleaved at each mt's stop
                qjs = deq(ws, zb, sb, 3, ci)
                if ci + 1 < NCH:
                    deq0 = deq(*chunks[ci + 1], 0, ci + 1)

                for mt in range(MT):
                    for a in range(A):
                        nc.tensor.matmul(
                            ps[mt][:],
                            xTs[6][:, a, mt * P : (mt + 1) * P],
                            qjs[0][:, a, :],
                            start=False,
                            stop=False,
                        )
                for mt in range(MT):
                    for a in range(A):
                        nc.tensor.matmul(
                            ps[mt][:],
                            xTs[7][:, a, mt * P : (mt + 1) * P],
                            qjs[1][:, a, :],
                            start=False,
                            stop=(a == A - 1),
                        )
                    osb = osb_pool.tile([P, nc_chunk], F32, tag="osb")
                    nc.vector.tensor_add(osb[:], ps[mt][:], res_tiles[mt][:])
                    nc.sync.dma_start(out[mt * P : (mt + 1) * P, nsl], osb[:])

    nc.compile()
    return nc


_NC_CACHE = {}


def _get_nc():
    if "nc" not in _NC_CACHE:
        _NC_CACHE["nc"] = build_nc()
    return _NC_CACHE["nc"]


def kernel(input, weight, weight_scales, weight_zeros, bias, residual, **run_kwargs):
    """Full-input entry point: shards across 8 NeuronCores, returns full output."""
    xtp, wsx, zbx, sbx, res = host_prep(
        input, weight, weight_scales, weight_zeros, bias, residual
    )
    nc = _get_nc()
    in_maps = []
    for i in range(N_CORES):
        rows = slice(i * M_SHARD, (i + 1) * M_SHARD)
        in_maps.append(
            {
                "xtp": np.ascontiguousarray(xtp[:, :, rows]),
                "wsx": wsx,
                "zbx": zbx,
                "sbx": sbx,
                "res": np.ascontiguousarray(res[rows]),
            }
        )
    result = run_bass_kernel_spmd(
        nc, in_maps, core_ids=list(range(N_CORES)), **run_kwargs
    )
    shards = [result.results[i]["out"] for i in range(N_CORES)]
    full = np.concatenate(shards, axis=0).reshape(B, S, N).astype(np.float32)
    if run_kwargs:
        return full, result
    return full


# revision 15
# speedup vs baseline: 1.0374x; 1.0149x over previous
"""GPTQ 4-bit fused dequant + GEMM + bias + residual for Trainium2 (Bass/Tile).

Problem: out[b,s,n] = sum_k x[b,s,k] * W[k,n] + bias[n] + residual[b,s,n]
  where W = (q - z) * s is 4-bit group-quantized (group size 128 along K),
  x: [4, 2048, 4096] f32, packed weight: [512, 4096] int32 (8 nibbles/word).

Sharding: data-parallel over rows (B*S = 8192 -> 1024 rows/core on 8 cores).
Each core reads its x/residual shard plus the (small, packed) full weight,
dequantizes W on-chip, and computes its output shard; no collectives.

The kernel is TensorE-bound: the bf16 GEMM floor is ~437 us/core and the
schedule keeps the PE array back-to-back (measured 216 ns per 128x128x512
matmul = the N/2.4GHz streaming floor). fp8 DoubleRow was tried and rejected:
quarter-K fp8 passes accuracy (1.67e-2 < 2e-2; more fp8 fails) but its power
draw clock-throttles the whole chip by 1.2x, a net loss at any allowed mix.

Host prep does all layout work so the device only streams:
  - x transposed/permuted/bf16-cast to [p, t, m] with k = 1024a + 8p + j for
    t = 8s + 4h + a (j = s + 4h), making the packed-word unpacking full-width
    with both matmul operands on the same k ordering; no on-chip transpose.
  - packed weights pre-split into u16 halves, chunk-major (fully contiguous
    per-chunk loads); scales/zeros broadcast to the partition layout in bf16.
  - bias folded into the residual (exact f32 add).

Scheduling: all input loads ride one in-order DMA ring so chunk-0's
weights/scales and the leading x pieces beat the bulk-x flood (short head);
residual loads go mid-chunk on the scalar ring; the DVE queue runs shifts +
sub/mul dequant with the next chunk's first group issued ahead of the
epilogue adds; j-outer/mt-inner matmul order staggers PSUM bank release at
chunk boundaries, with the per-mt epilogue interleaved into the final sweep.
"""

import numpy as np

import concourse.mybir as mybir
import concourse.tile as tile
from concourse import bacc
from concourse.bass_utils import run_bass_kernel_spmd

F32 = mybir.dt.float32
BF16 = mybir.dt.bfloat16
I32 = mybir.dt.int32
U16 = mybir.dt.uint16

P = 128  # partitions
JT = 8  # nibbles per int32
NIB = 4  # bits per nibble

# Full problem shape (hardcoded per harness contract)
B, S, K, N = 4, 2048, 4096, 4096
N_CORES = 8
M_FULL = B * S
M_SHARD = M_FULL // N_CORES


def host_prep(input, weight, weight_scales, weight_zeros, bias, residual,
              n=N, nc_chunk=512):
    """Host-side layout transforms (device streams these directly)."""
    import ml_dtypes

    BF = ml_dtypes.bfloat16
    A = (K // JT) // P  # 4
    NCH = n // nc_chunk

    # x[m, 1024a + 8p + j] -> xtp[p, 8s + 4h + a, m], j = s + 4h, bf16
    xf = np.asarray(input, dtype=np.float32).reshape(M_FULL, K)
    x5 = xf.reshape(M_FULL, A, P, 2, 4)
    xtp = x5.astype(BF).transpose(2, 4, 3, 1, 0)
    xtp = np.ascontiguousarray(xtp.reshape(P, JT * A, M_FULL))

    # packed words -> u16 halves, chunk-major so each chunk's load is fully
    # contiguous per partition: wsx[ci, p, h, a, nc] = half h of w[128a+p, n]
    w = np.ascontiguousarray(np.asarray(weight, dtype=np.int32))
    wsx = w.view("<u2").reshape(A, P, n, 2).transpose(1, 3, 0, 2)
    wsx = wsx.reshape(P, 2, A, NCH, nc_chunk).transpose(3, 0, 1, 2, 4)
    wsx = np.ascontiguousarray(wsx)

    # scales/zeros broadcast to [ci, p, a, nc]: zb[p, a, n] = z[8a + p//16, n]
    G = weight_scales.shape[0]
    AG = G // JT

    def bcast(t):
        r = t.reshape(AG, JT, n)
        r = np.repeat(r, 16, axis=1)
        r = r.transpose(1, 0, 2)
        r = r.reshape(P, AG, NCH, nc_chunk).transpose(2, 0, 1, 3)
        return np.ascontiguousarray(r.astype(BF))

    zbx = bcast(np.asarray(weight_zeros, dtype=np.float32))
    sbx = bcast(np.asarray(weight_scales, dtype=np.float32))

    # bias folded into residual (exact f32 add)
    res = np.asarray(residual, dtype=np.float32).reshape(M_FULL, n)
    res = res + np.asarray(bias, dtype=np.float32)[None, :]

    return xtp, wsx, zbx, sbx, np.ascontiguousarray(res)


def build_nc(m_shard=M_SHARD, k=K, n=N, nc_chunk=512):
    """Build the per-core Bass program (SPMD: same program on all cores)."""
    KP = k // JT  # packed rows (512)
    A = KP // P  # 128-row blocks of packed rows (4)
    MT = m_shard // P  # m tiles (8)
    NCH = n // nc_chunk  # n chunks (8)

    nc = bacc.Bacc("TRN2", target_bir_lowering=False)

    xtp = nc.dram_tensor("xtp", [P, JT * A, m_shard], BF16, kind="ExternalInput")
    wsx = nc.dram_tensor("wsx", [NCH, P, 2, A, nc_chunk], U16, kind="ExternalInput")
    zbx = nc.dram_tensor("zbx", [NCH, P, A, nc_chunk], BF16, kind="ExternalInput")
    sbx = nc.dram_tensor("sbx", [NCH, P, A, nc_chunk], BF16, kind="ExternalInput")
    res_in = nc.dram_tensor("res", [m_shard, n], F32, kind="ExternalInput")
    out = nc.dram_tensor("out", [m_shard, n], F32, kind="ExternalOutput")

    with tile.TileContext(nc) as tc:
        with (
            tc.tile_pool(name="persist", bufs=1) as persist,
            tc.tile_pool(name="ws", bufs=3) as ws_pool,
            tc.tile_pool(name="qs", bufs=4) as qs_pool,
            tc.tile_pool(name="q", bufs=6) as q_pool,
            tc.tile_pool(name="zs", bufs=3) as zs_pool,
            tc.tile_pool(name="res", bufs=12) as res_pool,
            tc.tile_pool(name="osb", bufs=3) as osb_pool,
            tc.tile_pool(name="psum", bufs=8, space="PSUM") as psum_pool,
        ):
            # ---- all input loads share the sync ring: in-order FIFO gives
            # chunk-0's weights/scales and the leading x pieces priority over
            # the bulk-x flood, so the first matmul starts early ----
            def load_chunk(ci):
                # u16 halves as separate tiles/DMAs: the first dequant group
                # (h=0) only waits on half the weight bytes
                wh = []
                for h in range(2):
                    w1 = ws_pool.tile([P, A, nc_chunk], U16, tag=f"ws{h}",
                                      name=f"ws{ci}_{h}")
                    nc.sync.dma_start(w1[:], wsx[ci][:, h])
                    if h == 0:
                        zb = zs_pool.tile([P, A, nc_chunk], BF16, tag="zb",
                                          name=f"zb{ci}")
                        sb = zs_pool.tile([P, A, nc_chunk], BF16, tag="sb",
                                          name=f"sb{ci}")
                        nc.sync.dma_start(zb[:], zbx[ci])
                        nc.sync.dma_start(sb[:], sbx[ci])
                    wh.append(w1)
                return wh[0], wh[1], zb, sb

            xTs = [
                persist.tile([P, A, m_shard], BF16, tag=f"xT{i}", name=f"xT{i}")
                for i in range(JT)
            ]

            chunks = {0: load_chunk(0)}
            # first x piece per-a so the very first matmul waits on 256 KB
            for a in range(A):
                nc.sync.dma_start(xTs[0][:, a, :], xtp[:, a : a + 1, :])
            nc.sync.dma_start(xTs[1][:], xtp[:, A : 2 * A, :])
            chunks[1] = load_chunk(1)
            for i in range(2, JT):
                nc.sync.dma_start(xTs[i][:], xtp[:, A * i : A * (i + 1), :])

            def deq(wh0, wh1, zb, sb, s, ci):
                # ((word >> 4s) & 15) per u16 half (the sub below casts
                # u16 -> bf16; bitwise TS ops cannot cast)
                qjs = []
                for h, wsh in ((0, wh0), (1, wh1)):
                    qsb = qs_pool.tile([P, A, nc_chunk], U16, tag="qs",
                                       name=f"qs{ci}_{s}_{h}")
                    nc.vector.tensor_scalar(
                        out=qsb[:],
                        in0=wsh[:],
                        scalar1=NIB * s,
                        scalar2=15,
                        op0=mybir.AluOpType.logical_shift_right,
                        op1=mybir.AluOpType.bitwise_and,
                    )
                    qj = q_pool.tile([P, A, nc_chunk], BF16, tag="q",
                                     name=f"q{ci}_{s}_{h}")
                    nc.vector.tensor_sub(qj[:], qsb[:], zb[:])
                    nc.vector.tensor_mul(qj[:], qj[:], sb[:])
                    qjs.append(qj)
                return qjs

            deq0 = deq(*chunks[0], 0, 0)

            for ci in range(NCH):
                nsl = slice(ci * nc_chunk, (ci + 1) * nc_chunk)
                wh0, wh1, zb, sb = chunks.pop(ci)
                if ci + 2 < NCH:
                    chunks[ci + 2] = load_chunk(ci + 2)

                ps = [
                    psum_pool.tile([P, nc_chunk], F32, tag="ps", name=f"ps{ci}_{mt}")
                    for mt in range(MT)
                ]
                res_tiles = []

                for s in range(3):
                    qjs = deq0 if s == 0 else deq(wh0, wh1, zb, sb, s, ci)
                    for h in range(2):
                        for mt in range(MT):
                            for a in range(A):
                                nc.tensor.matmul(
                                    ps[mt][:],
                                    xTs[2 * s + h][:, a, mt * P : (mt + 1) * P],
                                    qjs[h][:, a, :],
                                    start=(s == 0 and h == 0 and a == 0),
                                    stop=False,
                                )
                    if s >= 1:
                        # residual loads mid-chunk (scalar ring): off the
                        # head/boundary critical path, ready for the epilogue
                        for mt in range((s - 1) * MT // 2, s * MT // 2):
                            r = res_pool.tile([P, nc_chunk], F32, tag="res",
                                              name=f"res{ci}_{mt}")
                            nc.scalar.dma_start(
                                r[:], res_in[mt * P : (mt + 1) * P, nsl]
                            )
                            res_tiles.append(r)

                # last k-group: dequant, then next chunk's first dequant
                # (ahead of the epilogue adds in the DVE queue), then matmuls
                # with the per-mt epilogue interleaved at each mt's stop
                qjs = deq(wh0, wh1, zb, sb, 3, ci)
                if ci + 1 < NCH:
                    deq0 = deq(*chunks[ci + 1], 0, ci + 1)

                for mt in range(MT):
                    for a in range(A):
                        nc.tensor.matmul(
                            ps[mt][:],
                            xTs[6][:, a, mt * P : (mt + 1) * P],
                            qjs[0][:, a, :],
                            start=False,
                            stop=False,
                        )
                for mt in range(MT):
                    for a in range(A):
                        nc.tensor.matmul(
                            ps[mt][:],
                            xTs[7][:, a, mt * P : (mt + 1) * P],
                            qjs[1][:, a, :],
                            start=False,
                            stop=(a == A - 1),
                        )
                    osb = osb_pool.tile([P, nc_chunk], F32, tag="osb")
                    nc.vector.tensor_add(osb[:], ps[mt][:], res_tiles[mt][:])
                    nc.sync.dma_start(out[mt * P : (mt + 1) * P, nsl], osb[:])

    nc.compile()
    return nc


_NC_CACHE = {}


def _get_nc():
    if "nc" not in _NC_CACHE:
        _NC_CACHE["nc"] = build_nc()
    return _NC_CACHE["nc"]


def kernel(input, weight, weight_scales, weight_zeros, bias, residual, **run_kwargs):
    """Full-input entry point: shards across 8 NeuronCores, returns full output."""
    xtp, wsx, zbx, sbx, res = host_prep(
        input, weight, weight_scales, weight_zeros, bias, residual
    )
    nc = _get_nc()
    in_maps = []
    for i in range(N_CORES):
        rows = slice(i * M_SHARD, (i + 1) * M_SHARD)
        in_maps.append(
            {
                "xtp": np.ascontiguousarray(xtp[:, :, rows]),
                "wsx": wsx,
                "zbx": zbx,
                "sbx": sbx,
                "res": np.ascontiguousarray(res[rows]),
            }
        )
    result = run_bass_kernel_spmd(
        nc, in_maps, core_ids=list(range(N_CORES)), **run_kwargs
    )
    shards = [result.results[i]["out"] for i in range(N_CORES)]
    full = np.concatenate(shards, axis=0).reshape(B, S, N).astype(np.float32)
    if run_kwargs:
        return full, result
    return full


# revision 16
# speedup vs baseline: 1.0657x; 1.0273x over previous
"""GPTQ 4-bit fused dequant + GEMM + bias + residual for Trainium2 (Bass/Tile).

Problem: out[b,s,n] = sum_k x[b,s,k] * W[k,n] + bias[n] + residual[b,s,n]
  where W = (q - z) * s is 4-bit group-quantized (group size 128 along K),
  x: [4, 2048, 4096] f32, packed weight: [512, 4096] int32 (8 nibbles/word).

Sharding: data-parallel over rows (B*S = 8192 -> 1024 rows/core on 8 cores).
Each core reads its x/residual shard plus the (small, packed) full weight,
dequantizes W on-chip, and computes its output shard; no collectives.

The kernel is TensorE-bound: the bf16 GEMM floor is ~437 us/core and the
schedule keeps the PE array back-to-back (measured 216 ns per 128x128x512
matmul = the N/2.4GHz streaming floor). fp8 DoubleRow was tried and rejected:
quarter-K fp8 passes accuracy (1.67e-2 < 2e-2; more fp8 fails) but its power
draw clock-throttles the whole chip by 1.2x, a net loss at any allowed mix.

Host prep does all layout work so the device only streams:
  - x transposed/permuted/bf16-cast to [p, t, m] with k = 1024a + 8p + j for
    t = 8s + 4h + a (j = s + 4h), making the packed-word unpacking full-width
    with both matmul operands on the same k ordering; no on-chip transpose.
  - packed weights pre-split into u16 halves, chunk-major (fully contiguous
    per-chunk loads); scales/zeros broadcast to the partition layout in bf16.
  - bias folded into the residual (exact f32 add).

Scheduling: all input loads ride one in-order DMA ring so chunk-0's
weights/scales and the leading x pieces beat the bulk-x flood (short head);
residual loads go mid-chunk on the scalar ring; the DVE queue runs shifts +
sub/mul dequant with the next chunk's first group issued ahead of the
epilogue adds; j-outer/mt-inner matmul order staggers PSUM bank release at
chunk boundaries, with the per-mt epilogue interleaved into the final sweep.
"""

import numpy as np

import concourse.mybir as mybir
import concourse.tile as tile
from concourse import bacc
from concourse.bass_utils import run_bass_kernel_spmd

F32 = mybir.dt.float32
BF16 = mybir.dt.bfloat16
I32 = mybir.dt.int32
U16 = mybir.dt.uint16

P = 128  # partitions
JT = 8  # nibbles per int32
NIB = 4  # bits per nibble

# Full problem shape (hardcoded per harness contract)
B, S, K, N = 4, 2048, 4096, 4096
N_CORES = 8
M_FULL = B * S
M_SHARD = M_FULL // N_CORES


def host_prep(input, weight, weight_scales, weight_zeros, bias, residual,
              n=N, nc_chunk=512):
    """Host-side layout transforms (device streams these directly)."""
    import ml_dtypes

    BF = ml_dtypes.bfloat16
    A = (K // JT) // P  # 4
    NCH = n // nc_chunk

    # x[m, 1024a + 8p + j] -> xtp[p, 8s + 4h + a, m], j = s + 4h, bf16
    xf = np.asarray(input, dtype=np.float32).reshape(M_FULL, K)
    x5 = xf.reshape(M_FULL, A, P, 2, 4)
    xtp = x5.astype(BF).transpose(2, 4, 3, 1, 0)
    xtp = np.ascontiguousarray(xtp.reshape(P, JT * A, M_FULL))

    # packed words -> u16 halves, chunk-major so each chunk's load is fully
    # contiguous per partition: wsx[ci, p, h, a, nc] = half h of w[128a+p, n]
    w = np.ascontiguousarray(np.asarray(weight, dtype=np.int32))
    wsx = w.view("<u2").reshape(A, P, n, 2).transpose(1, 3, 0, 2)
    wsx = wsx.reshape(P, 2, A, NCH, nc_chunk).transpose(3, 0, 1, 2, 4)
    wsx = np.ascontiguousarray(wsx)

    # scales/zeros broadcast to [ci, p, a, nc]: zb[p, a, n] = z[8a + p//16, n]
    G = weight_scales.shape[0]
    AG = G // JT

    def bcast(t):
        r = t.reshape(AG, JT, n)
        r = np.repeat(r, 16, axis=1)
        r = r.transpose(1, 0, 2)
        r = r.reshape(P, AG, NCH, nc_chunk).transpose(2, 0, 1, 3)
        return np.ascontiguousarray(r.astype(BF))

    zbx = bcast(np.asarray(weight_zeros, dtype=np.float32))
    sbx = bcast(np.asarray(weight_scales, dtype=np.float32))

    # bias folded into residual (exact f32 add)
    res = np.asarray(residual, dtype=np.float32).reshape(M_FULL, n)
    res = res + np.asarray(bias, dtype=np.float32)[None, :]

    return xtp, wsx, zbx, sbx, np.ascontiguousarray(res)


def build_nc(m_shard=M_SHARD, k=K, n=N, nc_chunk=512):
    """Build the per-core Bass program (SPMD: same program on all cores)."""
    KP = k // JT  # packed rows (512)
    A = KP // P  # 128-row blocks of packed rows (4)
    MT = m_shard // P  # m tiles (8)
    NCH = n // nc_chunk  # n chunks (8)

    nc = bacc.Bacc("TRN2", target_bir_lowering=False)

    xtp = nc.dram_tensor("xtp", [P, JT * A, m_shard], BF16, kind="ExternalInput")
    wsx = nc.dram_tensor("wsx", [NCH, P, 2, A, nc_chunk], U16, kind="ExternalInput")
    zbx = nc.dram_tensor("zbx", [NCH, P, A, nc_chunk], BF16, kind="ExternalInput")
    sbx = nc.dram_tensor("sbx", [NCH, P, A, nc_chunk], BF16, kind="ExternalInput")
    res_in = nc.dram_tensor("res", [m_shard, n], F32, kind="ExternalInput")
    out = nc.dram_tensor("out", [m_shard, n], F32, kind="ExternalOutput")

    with tile.TileContext(nc) as tc:
        with (
            tc.tile_pool(name="persist", bufs=1) as persist,
            tc.tile_pool(name="ws", bufs=3) as ws_pool,
            tc.tile_pool(name="qs", bufs=4) as qs_pool,
            tc.tile_pool(name="q", bufs=6) as q_pool,
            tc.tile_pool(name="zs", bufs=3) as zs_pool,
            tc.tile_pool(name="res", bufs=12) as res_pool,
            tc.tile_pool(name="osb", bufs=3) as osb_pool,
            tc.tile_pool(name="psum", bufs=8, space="PSUM") as psum_pool,
        ):
            # ---- all input loads share the sync ring: in-order FIFO gives
            # chunk-0's weights/scales and the leading x pieces priority over
            # the bulk-x flood, so the first matmul starts early ----
            def load_chunk(ci):
                # u16 halves as separate tiles/DMAs: the first dequant group
                # (h=0) only waits on half the weight bytes
                wh = []
                for h in range(2):
                    w1 = ws_pool.tile([P, A, nc_chunk], U16, tag=f"ws{h}",
                                      name=f"ws{ci}_{h}")
                    nc.sync.dma_start(w1[:], wsx[ci][:, h])
                    if h == 0:
                        zb = zs_pool.tile([P, A, nc_chunk], BF16, tag="zb",
                                          name=f"zb{ci}")
                        sb = zs_pool.tile([P, A, nc_chunk], BF16, tag="sb",
                                          name=f"sb{ci}")
                        nc.sync.dma_start(zb[:], zbx[ci])
                        nc.sync.dma_start(sb[:], sbx[ci])
                    wh.append(w1)
                return wh[0], wh[1], zb, sb

            xTs = [
                persist.tile([P, A, m_shard], BF16, tag=f"xT{i}", name=f"xT{i}")
                for i in range(JT)
            ]

            chunks = {0: load_chunk(0)}
            # first x piece per-a so the very first matmul waits on 256 KB
            for a in range(A):
                nc.sync.dma_start(xTs[0][:, a, :], xtp[:, a : a + 1, :])
            nc.sync.dma_start(xTs[1][:], xtp[:, A : 2 * A, :])
            chunks[1] = load_chunk(1)
            for i in range(2, JT):
                nc.sync.dma_start(xTs[i][:], xtp[:, A * i : A * (i + 1), :])

            def deq(wh0, wh1, zb, sb, s, ci):
                # ((word >> 4s) & 15) per u16 half (the sub below casts
                # u16 -> bf16; bitwise TS ops cannot cast)
                qjs = []
                for h, wsh in ((0, wh0), (1, wh1)):
                    qsb = qs_pool.tile([P, A, nc_chunk], U16, tag="qs",
                                       name=f"qs{ci}_{s}_{h}")
                    nc.vector.tensor_scalar(
                        out=qsb[:],
                        in0=wsh[:],
                        scalar1=NIB * s,
                        scalar2=15,
                        op0=mybir.AluOpType.logical_shift_right,
                        op1=mybir.AluOpType.bitwise_and,
                    )
                    qj = q_pool.tile([P, A, nc_chunk], BF16, tag="q",
                                     name=f"q{ci}_{s}_{h}")
                    nc.vector.tensor_sub(qj[:], qsb[:], zb[:])
                    nc.vector.tensor_mul(qj[:], qj[:], sb[:])
                    qjs.append(qj)
                return qjs

            deq0 = deq(*chunks[0], 0, 0)

            for ci in range(NCH):
                nsl = slice(ci * nc_chunk, (ci + 1) * nc_chunk)
                wh0, wh1, zb, sb = chunks.pop(ci)
                if ci + 2 < NCH:
                    chunks[ci + 2] = load_chunk(ci + 2)

                ps = [
                    psum_pool.tile([P, nc_chunk], F32, tag="ps", name=f"ps{ci}_{mt}")
                    for mt in range(MT)
                ]
                res_tiles = []

                for s in range(3):
                    qjs = deq0 if s == 0 else deq(wh0, wh1, zb, sb, s, ci)
                    for h in range(2):
                        for mt in range(MT):
                            for a in range(A):
                                nc.tensor.matmul(
                                    ps[mt][:],
                                    xTs[2 * s + h][:, a, mt * P : (mt + 1) * P],
                                    qjs[h][:, a, :],
                                    start=(s == 0 and h == 0 and a == 0),
                                    stop=False,
                                )
                    if s >= 1:
                        # residual loads mid-chunk (scalar ring): off the
                        # head/boundary critical path, ready for the epilogue
                        for mt in range((s - 1) * MT // 2, s * MT // 2):
                            r = res_pool.tile([P, nc_chunk], F32, tag="res",
                                              name=f"res{ci}_{mt}")
                            nc.scalar.dma_start(
                                r[:], res_in[mt * P : (mt + 1) * P, nsl]
                            )
                            res_tiles.append(r)

                # last k-group: dequant, then next chunk's first dequant
                # (ahead of the epilogue adds in the DVE queue), then matmuls
                # with the per-mt epilogue interleaved at each mt's stop
                qjs = deq(wh0, wh1, zb, sb, 3, ci)
                if ci + 1 < NCH:
                    deq0 = deq(*chunks[ci + 1], 0, ci + 1)

                # mt-outer: each mt's last 8 k-tiles run consecutively, so
                # its stop lands up to ~12us before chunk end and the
                # epilogue (ADD + store) drains while later mts compute
                for mt in range(MT):
                    for h in range(2):
                        for a in range(A):
                            nc.tensor.matmul(
                                ps[mt][:],
                                xTs[6 + h][:, a, mt * P : (mt + 1) * P],
                                qjs[h][:, a, :],
                                start=False,
                                stop=(h == 1 and a == A - 1),
                            )
                    osb = osb_pool.tile([P, nc_chunk], F32, tag="osb")
                    nc.vector.tensor_add(osb[:], ps[mt][:], res_tiles[mt][:])
                    nc.sync.dma_start(out[mt * P : (mt + 1) * P, nsl], osb[:])

    nc.compile()
    return nc


_NC_CACHE = {}


def _get_nc():
    if "nc" not in _NC_CACHE:
        _NC_CACHE["nc"] = build_nc()
    return _NC_CACHE["nc"]


def kernel(input, weight, weight_scales, weight_zeros, bias, residual, **run_kwargs):
    """Full-input entry point: shards across 8 NeuronCores, returns full output."""
    xtp, wsx, zbx, sbx, res = host_prep(
        input, weight, weight_scales, weight_zeros, bias, residual
    )
    nc = _get_nc()
    in_maps = []
    for i in range(N_CORES):
        rows = slice(i * M_SHARD, (i + 1) * M_SHARD)
        in_maps.append(
            {
                "xtp": np.ascontiguousarray(xtp[:, :, rows]),
                "wsx": wsx,
                "zbx": zbx,
                "sbx": sbx,
                "res": np.ascontiguousarray(res[rows]),
            }
        )
    result = run_bass_kernel_spmd(
        nc, in_maps, core_ids=list(range(N_CORES)), **run_kwargs
    )
    shards = [result.results[i]["out"] for i in range(N_CORES)]
    full = np.concatenate(shards, axis=0).reshape(B, S, N).astype(np.float32)
    if run_kwargs:
        return full, result
    return full
